# revision 48
# baseline (speedup 1.0000x reference)
"""GCN encoder (2x GCNConv + MLP proj head) on 8 Trainium2 NeuronCores.

Strategy: shard nodes across the 8 cores (1250/core, padded to 1280).
The symmetric GCN norm dis[src]*dis[dst] factors into per-node pre/post
scaling, so each aggregation round is: per-core dense matmul (X@W, bf16,
f32 PSUM) + dis-scale -> AllGather of the scaled features -> per 128-dst
window: dma_gather of deduped source rows (2-chunk pieces across the 4
SWDGE queues, single_packet) + host-precomputed one-hot/count scatter
matmuls accumulating segment sums in PSUM -> self-loop term added on the
DVE (keeps the PE free) -> dis post-scale on the scalar engine.

Round 2 runs entirely in fp8e4: h2' is quantized once (out rel err
~1.8e-2 < the 2e-2 gate; z/proj unaffected), halving both h2 AllGathers
and the round-2 gather bytes, and the scatter matmuls use DoubleRow
perf mode (2 chunk-pairs per matmul at 2x PE rate) against the fp8 S
table. The S table ships as fp8 (counts are exact) and is DVE-cast to
bf16 per window for round 1. The proj head's matmuls are interleaved
into the round-2 windows where the tensor engine would otherwise idle
on gathers and the second h2 AllGather half.
"""
import json

import numpy as np
import ml_dtypes

N = 10000
E = 160000
D = 512
NC = 8
NPC = N // NC  # 1250 nodes per core
CH = 10  # 128-node chunks / windows per core
NPAD = CH * 128  # 1280

_BF16 = ml_dtypes.bfloat16
_F8 = ml_dtypes.float8_e4m3

_WAIT_SPLIT_DONE = False


def _install_wait_split():
    """This container's walrus rejects instructions with >1 sync wait.
    Hoist extra waits onto single-wait Drain instructions just before the
    instruction on the same engine (same sequencer => same semantics)."""
    global _WAIT_SPLIT_DONE
    if _WAIT_SPLIT_DONE:
        return
    _WAIT_SPLIT_DONE = True
    import concourse.bass as bass

    orig = bass.Bass.to_json_bytes

    def _split_block(instructions):
        out = []
        changed = False
        for inst in instructions:
            sync = inst.get("sync_info")
            waits = (sync or {}).get("on_wait") or []
            if len(waits) > 1:
                changed = True
                for j, w in enumerate(waits[:-1]):
                    out.append(
                        {
                            "engine": inst["engine"],
                            "ins": [],
                            "name": f"{inst['name']}-wsplit{j}",
                            "opcode": "Drain",
                            "outs": [],
                            "sync_info": {"on_update": [], "on_wait": [w]},
                        }
                    )
                sync["on_wait"] = waits[-1:]
            out.append(inst)
        return out, changed

    def to_json_bytes(self):
        js = json.loads(orig(self))
        stack = [js]
        while stack:
            d = stack.pop()
            if isinstance(d, dict):
                if "instructions" in d:
                    new, changed = _split_block(d["instructions"])
                    if changed:
                        d["instructions"] = new
                for v in d.values():
                    if isinstance(v, (dict, list)):
                        stack.append(v)
            elif isinstance(d, list):
                stack.extend(d)
        return json.dumps(js).encode()

    bass.Bass.to_json_bytes = to_json_bytes


def _split3(k_cw):
    """Round-1 split: 2-chunk (256-desc, 256KB) gather pieces. 4-chunk bf16
    pieces (512KB of 1KB rows) hit SWDGE data backpressure and regress;
    round 2's fp8 rows are half the bytes so it uses 4-chunk pieces (see
    splits2) for more slack in the 8-lane DMASW sem rotation."""
    out = [2] * (k_cw // 2)
    if k_cw % 2:
        out.append(1)
    return out


def _build_program(k_cw, nA_pieces, has_b1, has_b2, has_bp1, has_bp2):
    import concourse.bass as bass
    import concourse.tile as tile
    from concourse import mybir
    from concourse.library_config import mlp
    from concourse.library_overlay import lower_extended_insts
    from concourse.tile_rust import add_dep_helper

    f32 = mybir.dt.float32
    bf16 = mybir.dt.bfloat16
    f8 = mybir.dt.float8e4
    i16 = mybir.dt.int16
    ACTF = mybir.ActivationFunctionType

    nc = bass.Bass(num_swdge_queues=4)

    # ---- external inputs (per-core layouts prepared on host) ----
    xt_ext = nc.dram_tensor("xt", [128, 4 * NPAD], bf16, kind="ExternalInput")
    w_ext = {
        nm: nc.dram_tensor(nm, [128, 4 * D], bf16, kind="ExternalInput")
        for nm in ("w1t", "w2t", "wp1t", "wp2t")
    }
    dis_ext = nc.dram_tensor("dis", [128, CH], f32, kind="ExternalInput")
    idx_ext = nc.dram_tensor(
        "idx16", [128, CH * k_cw * 8], i16, kind="ExternalInput"
    )
    # S (one-hot counts) ships as fp8e4 (counts are small ints, exact):
    # round 2 consumes it directly (fp8 DoubleRow matmuls); round 1 casts
    # per-window to bf16 on the DVE. Halves the dominant input load.
    s_ext = nc.dram_tensor(
        "stab", [128, CH * k_cw * 128], f8, kind="ExternalInput"
    )
    ident_ext = nc.dram_tensor("ident", [128, 128], bf16, kind="ExternalInput")
    b_ext = {}
    for nm, has in (
        ("b1", has_b1),
        ("b2", has_b2),
        ("bp1", has_bp1),
        ("bp2", has_bp2),
    ):
        if has:
            b_ext[nm] = nc.dram_tensor(nm, [128, D], f32, kind="ExternalInput")

    # ---- external outputs ----
    # z in bf16: quantization ~2^-9 rel, far under the 2e-2 gate; halves the
    # z writeback and frees an SBUF tag.
    z_out = nc.dram_tensor("z", [NPAD, D], bf16, kind="ExternalOutput")
    out_out = nc.dram_tensor("agg", [NPAD, D], f32, kind="ExternalOutput")
    proj_out = nc.dram_tensor("proj", [NPAD, D], f32, kind="ExternalOutput")

    # ---- internal DRAM ----
    HALF = NPAD // 2  # 640
    h1p_sh = nc.dram_tensor("h1p_sh", [NPAD, D], bf16)
    h1p_full = nc.dram_tensor("h1p_full", [NC * NPAD, D], bf16, addr_space="Shared")
    # round-2 features travel as fp8e4: halves the h2 AllGathers and the
    # round-2 gather traffic, and enables DoubleRow (2x) scatter matmuls.
    # Empirically out rel err ~1.8e-2 < 2e-2 gate (z/proj unaffected).
    h2p_sh = nc.dram_tensor("h2p_sh", [NPAD, D], f8)
    h2p_full = nc.dram_tensor("h2p_full", [NC * NPAD, D], f8, addr_space="Shared")

    core_ids = list(range(NC))
    splits = _split3(k_cw)
    # round 2: 4-chunk fp8 pieces -- halves the SWDGE DMA count per window,
    # doubling the slack in the 8-lane DMASW sem rotation (lane reuse is
    # fully serialized: desc-gen of a lane's next DMA waits for the lane's
    # previous DMA to complete)
    splits2 = [4] * (k_cw // 4)
    if k_cw % 4:
        splits2.append(k_cw % 4)

    with tile.TileContext(nc) as tc:
        with (
            tc.tile_pool(name="const", bufs=1) as cpool,
            tc.tile_pool(name="work", bufs=3) as wpool,
            tc.tile_pool(name="gat", bufs=4) as gpool,
            tc.tile_pool(name="sc", bufs=3) as scpool,
            tc.tile_pool(name="tp", bufs=1) as tpool,
            tc.tile_pool(name="psA", bufs=2, space="PSUM") as psA,
            tc.tile_pool(name="psB", bufs=4, space="PSUM") as psB,
        ):
            lib_inst = nc.gpsimd.load_library(mlp)
            # one shared register per distinct gather size (to_reg per call
            # would exhaust the Pool register file at 60 gathers)
            nidx_regs = {
                nk: nc.gpsimd.to_reg(nk * 128)
                for nk in sorted(set(splits) | set(splits2))
            }

            # ---- phase-critical constant loads (sync/SP HWDGE ring) ----
            # w1t first, then xt per-window so phase A pipelines with the
            # input loads instead of waiting for the whole xt block.
            w_t = {}
            w_t["w1t"] = cpool.tile([128, 4 * D], bf16, tag="w1t", name="w1t")
            nc.sync.dma_start(w_t["w1t"][:], w_ext["w1t"][:])
            dis_t = cpool.tile([128, CH], f32)
            nc.sync.dma_start(dis_t[:], dis_ext[:])
            xt_t = cpool.tile([128, 4 * NPAD], bf16)
            xt_v = xt_t[:].rearrange("p (k n) -> p k n", n=NPAD)
            xe_v = xt_ext[:].rearrange("p (k n) -> p k n", n=NPAD)
            for m in range(CH):
                nc.sync.dma_start(
                    xt_v[:, :, m * 128 : (m + 1) * 128],
                    xe_v[:, :, m * 128 : (m + 1) * 128],
                )
            ident_t = cpool.tile([128, 128], bf16)
            nc.sync.dma_start(ident_t[:], ident_ext[:])
            for nm in ("w2t", "wp1t", "wp2t"):
                w_t[nm] = cpool.tile([128, 4 * D], bf16, tag=nm, name=nm)
                nc.sync.dma_start(w_t[nm][:], w_ext[nm][:])
            b_t = {}
            for nm in b_ext:
                b_t[nm] = cpool.tile([128, D], f32, tag=nm, name=nm + "_bc")
                nc.sync.dma_start(b_t[nm][:], b_ext[nm][:])
            # ---- bulk loads on the scalar HWDGE ring (parallel to above) ----
            idx_t = cpool.tile([128, CH * k_cw * 8], i16)
            nc.scalar.dma_start(idx_t[:], idx_ext[:])
            s8_t = cpool.tile([128, CH * k_cw * 128], f8)
            nc.scalar.dma_start(s8_t[:], s_ext[:])

            # persistent scaled-feature chunks (self-loop term source)
            h1p_t = cpool.tile([128, CH * D], bf16)
            h2p_t = cpool.tile([128, CH * D], bf16)

            def dense_layer(lhs_tiles, w_name, m, dep=None):
                ps = psA.tile([128, D], f32, tag="dense")
                for kk in range(4):
                    mi = nc.tensor.matmul(
                        ps[:],
                        lhs_tiles(kk, m),
                        w_t[w_name][:, kk * D : (kk + 1) * D],
                        start=(kk == 0),
                        stop=(kk == 3),
                    )
                    if kk == 0 and dep is not None:
                        add_dep_helper(
                            getattr(mi, "ins", mi),
                            getattr(dep, "ins", dep),
                            reason="pin proj head to its window",
                        )
                return ps

            def xt_tile(kk, m):
                return xt_t[:, kk * NPAD + m * 128 : kk * NPAD + (m + 1) * 128]

            def scale_to(dst_ap, ps, m, bias_name):
                """dst = dis_m * (ps + bias) via ACT (bias pre-add on DVE).
                Returns the pre-scale source for further ACT copies."""
                if bias_name in b_t:
                    tmp = wpool.tile([128, D], f32, tag="btmp")
                    nc.vector.tensor_tensor(
                        tmp[:], ps[:], b_t[bias_name][:], op=mybir.AluOpType.add
                    )
                    src = tmp
                else:
                    src = ps
                nc.scalar.activation(
                    dst_ap, src[:], ACTF.Copy, scale=dis_t[:, m : m + 1]
                )
                return src

            # ---- phase A: H1' = dis * (X @ W1 + b1), own nodes ----
            for m in range(CH):
                ps = dense_layer(xt_tile, "w1t", m)
                scale_to(h1p_t[:, m * D : (m + 1) * D], ps, m, "b1")
                nc.sync.dma_start(
                    h1p_sh[m * 128 : (m + 1) * 128, :],
                    h1p_t[:, m * D : (m + 1) * D],
                )
                if m == 4:
                    nc.gpsimd.collective_compute(
                        "AllGather",
                        mybir.AluOpType.bypass,
                        ins=[h1p_sh[0:HALF, :]],
                        outs=[h1p_full[0 : NC * HALF, :]],
                        replica_groups=[core_ids],
                    )
            nc.gpsimd.collective_compute(
                "AllGather",
                mybir.AluOpType.bypass,
                ins=[h1p_sh[HALF:NPAD, :]],
                outs=[h1p_full[NC * HALF : 2 * NC * HALF, :]],
                replica_groups=[core_ids],
            )

            def one_gather(src_ap, w, a, nk, g_ap, qn):
                """Gather chunks [a, a+nk) of window w's table into g_ap."""
                gi = nc.gpsimd.dma_gather(
                    g_ap,
                    src_ap,
                    idx_t[
                        :,
                        (w * k_cw + a) * 8 : (w * k_cw + a + nk) * 8,
                    ],
                    num_idxs=nk * 128,
                    num_idxs_reg=nidx_regs[nk],
                    elem_size=D,
                    single_packet=True,
                    queue_num=qn,
                )
                add_dep_helper(
                    getattr(gi, "ins", gi),
                    getattr(lib_inst, "ins", lib_inst),
                    reason="mlp library before dma_gather",
                )

            def issue_gathers(src_full, w, g_t, pieces):
                a = 0
                for hh, nk in enumerate(pieces):
                    one_gather(src_full[:], w, a, nk, g_t[:, a : a + nk, :], hh % 4)
                    a += nk

            def self_add(ps, selfsrc_t, w):
                """Self-loop term on DVE (frees the PE of identity matmuls)."""
                tmp = wpool.tile([128, D], f32, tag="selfadd", bufs=3)
                nc.vector.tensor_tensor(
                    tmp[:],
                    ps[:],
                    selfsrc_t[:, w * D : (w + 1) * D],
                    op=mybir.AluOpType.add,
                )
                return tmp

            def agg_round1(src_full, selfsrc_t):
                """bf16 scatter: per window, DVE-cast the fp8 S chunk to bf16
                then accumulate k_cw chunk matmuls."""
                for w in range(CH):
                    g_t = gpool.tile([128, k_cw, D], bf16, tag="g", name=f"g{w}")
                    issue_gathers(src_full, w, g_t, splits)
                    s_c = scpool.tile(
                        [128, k_cw * 128], bf16, tag="sc", name=f"sc{w}"
                    )
                    nc.vector.tensor_copy(
                        s_c[:], s8_t[:, w * k_cw * 128 : (w + 1) * k_cw * 128]
                    )
                    ps = psB.tile([128, D], f32, tag="agg", name=f"agg{w}")
                    for k in range(k_cw):
                        nc.tensor.matmul(
                            ps[:],
                            s_c[:, k * 128 : (k + 1) * 128],
                            g_t[:, k, :],
                            start=(k == 0),
                            stop=(k == k_cw - 1),
                        )
                    yield w, self_add(ps, selfsrc_t, w)

            s8_pairs = s8_t[:].rearrange("p (c k) -> p c k", k=128)

            def agg_round2(src_full, selfsrc_t, pre=None):
                """fp8 DoubleRow scatter: pairs of 128-src chunks per matmul
                at 2x PE rate; gathers move half the bytes."""
                for w in range(CH):
                    # fp8 tile sized like the bf16 round-1 tiles (2*k_cw
                    # chunks) so both rounds share one pool tag; only the
                    # first k_cw chunks are used.
                    g_full = gpool.tile(
                        [128, 2 * k_cw, D], f8, tag="g", name=f"g8_{w}"
                    )
                    g_t = g_full[:, :k_cw, :]
                    issue_gathers(src_full, w, g_t, splits2)
                    if pre is not None:
                        pre(w)
                    ps = psB.tile([128, D], f32, tag="agg", name=f"agg{w}")
                    np_ = k_cw // 2
                    for c in range(np_):
                        mi = nc.tensor.matmul(
                            ps[:],
                            s8_pairs[:, w * k_cw + 2 * c : w * k_cw + 2 * c + 2, :],
                            g_t[:, 2 * c : 2 * c + 2, :],
                            start=(c == 0),
                            stop=(c == np_ - 1),
                            perf_mode=mybir.MatmulPerfMode.DoubleRow,
                        )
                    # anchor: proj of the NEXT window is pinned behind this
                    # window's agg so the scheduler can't hoist the whole proj
                    # head ahead of round 2 (which starves the gather pacing
                    # sems of matmul-count progress)
                    anchor[0] = mi
                    yield w, self_add(ps, selfsrc_t, w)

            # ---- fused round 1 + L2, pipelined per window ----
            zt_t = tpool.tile([128, 4 * NPAD], bf16, tag="zt")
            rt_t = tpool.tile([128, 4 * NPAD], bf16, tag="rt")

            def zt_tile(kk, m):
                return zt_t[:, kk * NPAD + m * 128 : kk * NPAD + (m + 1) * 128]

            def rt_tile(kk, m):
                return rt_t[:, kk * NPAD + m * 128 : kk * NPAD + (m + 1) * 128]

            for w, tmp in agg_round1(h1p_full, h1p_t):
                z_b = wpool.tile([128, D], bf16, tag="zb")
                nc.scalar.activation(
                    z_b[:], tmp[:], ACTF.Copy, scale=dis_t[:, w : w + 1]
                )
                nc.sync.dma_start(z_out[w * 128 : (w + 1) * 128, :], z_b[:])
                # transpose z chunk into zt columns; relu'd copy into rt
                psT = psA.tile([128, 4, 128], bf16, tag="tr", name=f"tr{w}")
                for kk in range(4):
                    nc.tensor.transpose(
                        psT[:, kk, :],
                        z_b[:, kk * 128 : (kk + 1) * 128],
                        ident_t[:],
                    )
                zt_cols = zt_t[:].rearrange("p (k n) -> p k n", n=NPAD)[
                    :, :, w * 128 : (w + 1) * 128
                ]
                rt_cols = rt_t[:].rearrange("p (k n) -> p k n", n=NPAD)[
                    :, :, w * 128 : (w + 1) * 128
                ]
                nc.vector.tensor_copy(zt_cols, psT[:])
                nc.vector.tensor_scalar(
                    rt_cols, psT[:], 0.0, None, op0=mybir.AluOpType.max
                )
                # L2 for this node chunk -> H2' shard (bf16 self copy + fp8
                # collective copy)
                ps2 = dense_layer(rt_tile, "w2t", w)
                src2 = scale_to(h2p_t[:, w * D : (w + 1) * D], ps2, w, "b2")
                h2f8 = wpool.tile([128, D], f8, tag="h2f8")
                nc.scalar.activation(
                    h2f8[:], src2[:], ACTF.Copy, scale=dis_t[:, w : w + 1]
                )
                nc.sync.dma_start(
                    h2p_sh[w * 128 : (w + 1) * 128, :], h2f8[:]
                )
                if w == 4:
                    nc.gpsimd.collective_compute(
                        "AllGather",
                        mybir.AluOpType.bypass,
                        ins=[h2p_sh[0:HALF, :]],
                        outs=[h2p_full[0 : NC * HALF, :]],
                        replica_groups=[core_ids],
                    )
            nc.gpsimd.collective_compute(
                "AllGather",
                mybir.AluOpType.bypass,
                ins=[h2p_sh[HALF:NPAD, :]],
                outs=[h2p_full[NC * HALF : 2 * NC * HALF, :]],
                replica_groups=[core_ids],
            )

            # ---- phase E: round 2 -> out, proj head interleaved per window
            # (proj depends only on zt, so its matmuls fill the tensor engine
            # while the h2 AllGather half and the window's gathers land) ----
            anchor = [None]

            def proj_head(w):
                ps3 = dense_layer(zt_tile, "wp1t", w, dep=anchor[0])
                p1_b = wpool.tile([128, D], bf16, tag="p1")
                if "bp1" in b_t:
                    btmp = wpool.tile([128, D], f32, tag="btmp")
                    nc.vector.tensor_tensor(
                        btmp[:], ps3[:], b_t["bp1"][:], op=mybir.AluOpType.add
                    )
                    nc.vector.tensor_scalar(
                        p1_b[:], btmp[:], 0.0, None, op0=mybir.AluOpType.max
                    )
                else:
                    nc.vector.tensor_scalar(
                        p1_b[:], ps3[:], 0.0, None, op0=mybir.AluOpType.max
                    )
                psT2 = psA.tile([128, 4, 128], bf16, tag="tr", name=f"tr2{w}")
                for kk in range(4):
                    nc.tensor.transpose(
                        psT2[:, kk, :],
                        p1_b[:, kk * 128 : (kk + 1) * 128],
                        ident_t[:],
                    )
                # p1^T is consumed immediately by proj2 — rotating tile, not
                # a persistent NPAD-wide buffer
                p1c = scpool.tile([128, 4, 128], bf16, tag="p1c", name=f"p1c{w}")
                nc.vector.tensor_copy(p1c[:], psT2[:])
                ps4 = dense_layer(lambda kk, m: p1c[:, kk, :], "wp2t", w)
                pj_t = wpool.tile([128, D], f32, tag="pj")
                if "bp2" in b_t:
                    nc.vector.tensor_tensor(
                        pj_t[:], ps4[:], b_t["bp2"][:], op=mybir.AluOpType.add
                    )
                else:
                    nc.vector.tensor_copy(pj_t[:], ps4[:])
                nc.sync.dma_start(proj_out[w * 128 : (w + 1) * 128, :], pj_t[:])

            for w, tmp in agg_round2(h2p_full, h2p_t, pre=proj_head):
                o_f = wpool.tile([128, D], f32, tag="of")
                nc.scalar.activation(
                    o_f[:], tmp[:], ACTF.Copy, scale=dis_t[:, w : w + 1]
                )
                nc.sync.dma_start(out_out[w * 128 : (w + 1) * 128, :], o_f[:])

    lower_extended_insts(nc)
    return nc


def _host_prep(x, edge_index, W1, W2, Wp1, Wp2):
    src = np.asarray(edge_index[0], np.int64)
    dst = np.asarray(edge_index[1], np.int64)

    # degree includes self loops (norm definition), but self edges are
    # handled on-device via the identity matmul, not the gather.
    deg = (np.bincount(np.concatenate([dst, np.arange(N)]), minlength=N)).astype(
        np.float32
    )
    dis = (1.0 / np.sqrt(np.maximum(deg, 1.0))).astype(np.float32)

    owner = src // NPC
    local = src - owner * NPC
    HALF = NPAD // 2
    # AllGather halves land rank-major per half: [8*640 | 8*640]
    gather_row = np.where(
        local < HALF,
        owner * HALF + local,
        NC * HALF + owner * HALF + (local - HALF),
    )

    dst_core = dst // NPC
    dst_local = dst - dst_core * NPC  # [0, 1250)
    win = dst_local // 128
    dloc = dst_local - win * 128

    order = np.lexsort((dst_local, dst_core))
    g_sorted = gather_row[order]
    dc = dst_core[order]
    wn = win[order]
    dl = dloc[order]

    counts = np.zeros((NC, CH), np.int64)
    np.add.at(counts, (dc, wn), 1)
    flat_counts = counts.reshape(-1)
    starts = np.concatenate([[0], np.cumsum(flat_counts)])[:-1].reshape(NC, CH)

    # dedup per (core, window); k_cw from max unique count
    uniq = {}
    max_u = 0
    for c in range(NC):
        for w in range(CH):
            s0, n = starts[c, w], counts[c, w]
            rows = g_sorted[s0 : s0 + n]
            dd = dl[s0 : s0 + n]
            u, inv = np.unique(rows, return_inverse=True)
            uniq[(c, w)] = (u, inv, dd)
            max_u = max(max_u, len(u))
    k_cw = int(np.ceil(max_u / 128))
    if k_cw % 2:
        k_cw += 1  # even: round-2 fp8 DoubleRow consumes chunk pairs
    wlen = k_cw * 128

    # Unique sources are sorted ascending, and AllGather half A lands in
    # rows [0, NC*HALF): the first cntA entries of each window's table
    # depend only on AG half A. Per window, the number of 2-chunk gather
    # pieces that are A-only on EVERY core (shared program) lets round-1
    # start those pieces as soon as AG0 lands (overlapping AG1).
    HALF = NPAD // 2
    nA_pieces = []
    for w in range(CH):
        m = min(
            int(np.searchsorted(uniq[(c, w)][0], NC * HALF)) for c in range(NC)
        )
        nA_pieces.append(min(m // 256, k_cw // 2 - 1))

    per_core = []
    for c in range(NC):
        idx_pad = np.zeros((CH, wlen), np.int64)
        s_tab = np.zeros((CH, wlen, 128), np.float32)
        for w in range(CH):
            u, inv, dd = uniq[(c, w)]
            idx_pad[w, : len(u)] = u
            np.add.at(s_tab[w], (inv, dd), 1.0)

        iw = idx_pad.reshape(CH, wlen // 16, 16).transpose(0, 2, 1)
        idx16 = np.tile(iw, (1, 8, 1)).transpose(1, 0, 2).reshape(128, -1)
        idx16 = np.ascontiguousarray(idx16, np.int16)

        # stab: [128, CH*k_cw*128]; col (w*k_cw+k)*128+d, part p = S[w, k*128+p, d]
        # fp8e4: counts are small ints, exactly representable
        stab = (
            s_tab.reshape(CH, k_cw, 128, 128)
            .transpose(2, 0, 1, 3)
            .reshape(128, -1)
        )
        stab = np.ascontiguousarray(stab).astype(_F8)

        xc = np.zeros((NPAD, D), np.float32)
        xc[:NPC] = x[c * NPC : (c + 1) * NPC]
        xt = xc.T.reshape(4, 128, NPAD).transpose(1, 0, 2).reshape(128, -1)
        xt = np.ascontiguousarray(xt).astype(_BF16)

        dis_c = np.zeros((NPAD,), np.float32)
        dis_c[:NPC] = dis[c * NPC : (c + 1) * NPC]
        dis_t = np.ascontiguousarray(dis_c.reshape(CH, 128).T, np.float32)

        per_core.append(
            {"xt": xt, "idx16": idx16, "stab": stab, "dis": dis_t}
        )

    def wtile(W):
        wt = (
            np.asarray(W, np.float32)
            .reshape(4, 128, D)
            .transpose(1, 0, 2)
            .reshape(128, -1)
        )
        return np.ascontiguousarray(wt).astype(_BF16)

    shared = {
        "w1t": wtile(W1),
        "w2t": wtile(W2),
        "wp1t": wtile(Wp1),
        "wp2t": wtile(Wp2),
        "ident": np.eye(128, dtype=np.float32).astype(_BF16),
    }
    return k_cw, nA_pieces, per_core, shared


def run(inputs, trace=False, **run_kwargs):
    """Build + run; returns ((out, z, proj), BassKernelResults)."""
    _install_wait_split()
    from concourse.bass_utils import run_bass_kernel_spmd

    x = np.asarray(inputs["x"], np.float32)
    b1, b2 = inputs["b1"], inputs["b2"]
    bp1, bp2 = inputs["bp1"], inputs["bp2"]
    k_cw, nA_pieces, per_core, shared = _host_prep(
        x, inputs["edge_index"], inputs["W1"], inputs["W2"], inputs["Wp1"],
        inputs["Wp2"],
    )

    has_b = {
        "b1": bool(np.any(np.asarray(b1))),
        "b2": bool(np.any(np.asarray(b2))),
        "bp1": bool(np.any(np.asarray(bp1))),
        "bp2": bool(np.any(np.asarray(bp2))),
    }
    nc = _build_program(
        k_cw, nA_pieces, has_b["b1"], has_b["b2"], has_b["bp1"], has_b["bp2"]
    )

    in_maps = []
    for c in range(NC):
        m = dict(per_core[c])
        m.update(shared)
        for nm, b in (("b1", b1), ("b2", b2), ("bp1", bp1), ("bp2", bp2)):
            if has_b[nm]:
                m[nm] = np.ascontiguousarray(
                    np.tile(np.asarray(b, np.float32)[None, :], (128, 1))
                )
        in_maps.append(m)

    res = run_bass_kernel_spmd(
        nc, in_maps, core_ids=list(range(NC)), trace=trace, **run_kwargs
    )

    out = np.empty((N, D), np.float32)
    z = np.empty((N, D), np.float32)
    proj = np.empty((N, D), np.float32)
    for c in range(NC):
        r = res.results[c]
        out[c * NPC : (c + 1) * NPC] = r["agg"][:NPC]
        z[c * NPC : (c + 1) * NPC] = r["z"][:NPC]
        proj[c * NPC : (c + 1) * NPC] = r["proj"][:NPC]
    return (out, z, proj), res


def kernel(x, edge_index, W1, b1, W2, b2, Wp1, bp1, Wp2, bp2):
    outs, _ = run(
        {
            "x": x, "edge_index": edge_index, "W1": W1, "b1": b1,
            "W2": W2, "b2": b2, "Wp1": Wp1, "bp1": bp1,
            "Wp2": Wp2, "bp2": bp2,
        }
    )
    return outs



# revision 49
# speedup vs baseline: 1.0437x; 1.0437x over previous
"""GCN encoder (2x GCNConv + MLP proj head) on 8 Trainium2 NeuronCores.

Strategy: shard nodes across the 8 cores (1250/core, padded to 1280).
The symmetric GCN norm dis[src]*dis[dst] factors into per-node pre/post
scaling, so each aggregation round is: per-core dense matmul (X@W, bf16,
f32 PSUM) + dis-scale -> AllGather of the scaled features -> per 128-dst
window: dma_gather of deduped source rows (2-chunk pieces across the 4
SWDGE queues, single_packet) + host-precomputed one-hot/count scatter
matmuls accumulating segment sums in PSUM -> self-loop term added on the
DVE (keeps the PE free) -> dis post-scale on the scalar engine.

Round 2 runs entirely in fp8e4: h2' is quantized once (out rel err
~1.8e-2 < the 2e-2 gate; z/proj unaffected), halving both h2 AllGathers
and the round-2 gather bytes, and the scatter matmuls use DoubleRow
perf mode (2 chunk-pairs per matmul at 2x PE rate) against the fp8 S
table. The S table ships as fp8 (counts are exact) and is DVE-cast to
bf16 per window for round 1. The proj head's matmuls are interleaved
into the round-2 windows where the tensor engine would otherwise idle
on gathers and the second h2 AllGather half.
"""
import json

import numpy as np
import ml_dtypes

N = 10000
E = 160000
D = 512
NC = 8
NPC = N // NC  # 1250 nodes per core
CH = 10  # 128-node chunks / windows per core
NPAD = CH * 128  # 1280

_BF16 = ml_dtypes.bfloat16
_F8 = ml_dtypes.float8_e4m3

_WAIT_SPLIT_DONE = False


def _install_wait_split():
    """This container's walrus rejects instructions with >1 sync wait.
    Hoist extra waits onto single-wait Drain instructions just before the
    instruction on the same engine (same sequencer => same semantics)."""
    global _WAIT_SPLIT_DONE
    if _WAIT_SPLIT_DONE:
        return
    _WAIT_SPLIT_DONE = True
    import concourse.bass as bass

    orig = bass.Bass.to_json_bytes

    def _split_block(instructions):
        out = []
        changed = False
        for inst in instructions:
            sync = inst.get("sync_info")
            waits = (sync or {}).get("on_wait") or []
            if len(waits) > 1:
                changed = True
                for j, w in enumerate(waits[:-1]):
                    out.append(
                        {
                            "engine": inst["engine"],
                            "ins": [],
                            "name": f"{inst['name']}-wsplit{j}",
                            "opcode": "Drain",
                            "outs": [],
                            "sync_info": {"on_update": [], "on_wait": [w]},
                        }
                    )
                sync["on_wait"] = waits[-1:]
            out.append(inst)
        return out, changed

    def to_json_bytes(self):
        js = json.loads(orig(self))
        stack = [js]
        while stack:
            d = stack.pop()
            if isinstance(d, dict):
                if "instructions" in d:
                    new, changed = _split_block(d["instructions"])
                    if changed:
                        d["instructions"] = new
                for v in d.values():
                    if isinstance(v, (dict, list)):
                        stack.append(v)
            elif isinstance(d, list):
                stack.extend(d)
        return json.dumps(js).encode()

    bass.Bass.to_json_bytes = to_json_bytes


def _split3(k_cw):
    """Round-1 split: 2-chunk (256-desc, 256KB) gather pieces. 4-chunk bf16
    pieces (512KB of 1KB rows) hit SWDGE data backpressure and regress;
    round 2's fp8 rows are half the bytes so it uses 4-chunk pieces (see
    splits2) for more slack in the 8-lane DMASW sem rotation."""
    out = [2] * (k_cw // 2)
    if k_cw % 2:
        out.append(1)
    return out


def _build_program(k_cw, nA_pieces, has_b1, has_b2, has_bp1, has_bp2):
    import concourse.bass as bass
    import concourse.tile as tile
    from concourse import mybir
    from concourse.library_config import mlp
    from concourse.library_overlay import lower_extended_insts
    from concourse.tile_rust import add_dep_helper

    f32 = mybir.dt.float32
    bf16 = mybir.dt.bfloat16
    f8 = mybir.dt.float8e4
    i16 = mybir.dt.int16
    ACTF = mybir.ActivationFunctionType

    nc = bass.Bass(num_swdge_queues=4)

    # ---- external inputs (per-core layouts prepared on host) ----
    xt_ext = nc.dram_tensor("xt", [128, 4 * NPAD], bf16, kind="ExternalInput")
    w_ext = {
        nm: nc.dram_tensor(nm, [128, 4 * D], bf16, kind="ExternalInput")
        for nm in ("w1t", "w2t", "wp1t", "wp2t")
    }
    dis_ext = nc.dram_tensor("dis", [128, CH], f32, kind="ExternalInput")
    idx_ext = nc.dram_tensor(
        "idx16", [128, CH * k_cw * 8], i16, kind="ExternalInput"
    )
    # S (one-hot counts) ships as fp8e4 (counts are small ints, exact):
    # round 2 consumes it directly (fp8 DoubleRow matmuls); round 1 casts
    # per-window to bf16 on the DVE. Halves the dominant input load.
    s_ext = nc.dram_tensor(
        "stab", [128, CH * k_cw * 128], f8, kind="ExternalInput"
    )
    ident_ext = nc.dram_tensor("ident", [128, 128], bf16, kind="ExternalInput")
    b_ext = {}
    for nm, has in (
        ("b1", has_b1),
        ("b2", has_b2),
        ("bp1", has_bp1),
        ("bp2", has_bp2),
    ):
        if has:
            b_ext[nm] = nc.dram_tensor(nm, [128, D], f32, kind="ExternalInput")

    # ---- external outputs ----
    # z in bf16: quantization ~2^-9 rel, far under the 2e-2 gate; halves the
    # z writeback and frees an SBUF tag.
    z_out = nc.dram_tensor("z", [NPAD, D], bf16, kind="ExternalOutput")
    out_out = nc.dram_tensor("agg", [NPAD, D], f32, kind="ExternalOutput")
    proj_out = nc.dram_tensor("proj", [NPAD, D], f32, kind="ExternalOutput")

    # ---- internal DRAM ----
    HALF = NPAD // 2  # 640
    h1p_sh = nc.dram_tensor("h1p_sh", [NPAD, D], bf16)
    h1p_full = nc.dram_tensor("h1p_full", [NC * NPAD, D], bf16, addr_space="Shared")
    # round-2 features travel as fp8e4: halves the h2 AllGathers and the
    # round-2 gather traffic, and enables DoubleRow (2x) scatter matmuls.
    # Empirically out rel err ~1.8e-2 < 2e-2 gate (z/proj unaffected).
    h2p_sh = nc.dram_tensor("h2p_sh", [NPAD, D], f8)
    h2p_full = nc.dram_tensor("h2p_full", [NC * NPAD, D], f8, addr_space="Shared")

    core_ids = list(range(NC))
    splits = _split3(k_cw)
    # round 2: 4-chunk fp8 pieces -- halves the SWDGE DMA count per window,
    # doubling the slack in the 8-lane DMASW sem rotation (lane reuse is
    # fully serialized: desc-gen of a lane's next DMA waits for the lane's
    # previous DMA to complete)
    splits2 = [4] * (k_cw // 4)
    if k_cw % 4:
        splits2.append(k_cw % 4)

    with tile.TileContext(nc) as tc:
        with (
            tc.tile_pool(name="const", bufs=1) as cpool,
            tc.tile_pool(name="work", bufs=3) as wpool,
            tc.tile_pool(name="gat", bufs=4) as gpool,
            tc.tile_pool(name="sc", bufs=3) as scpool,
            tc.tile_pool(name="tp", bufs=1) as tpool,
            tc.tile_pool(name="psA", bufs=2, space="PSUM") as psA,
            tc.tile_pool(name="psB", bufs=4, space="PSUM") as psB,
        ):
            lib_inst = nc.gpsimd.load_library(mlp)
            # one shared register per distinct gather size (to_reg per call
            # would exhaust the Pool register file at 60 gathers)
            nidx_regs = {
                nk: nc.gpsimd.to_reg(nk * 128)
                for nk in sorted(set(splits) | set(splits2))
            }

            # ---- phase-critical constant loads (sync/SP HWDGE ring) ----
            # w1t first, then xt per-window so phase A pipelines with the
            # input loads instead of waiting for the whole xt block.
            w_t = {}
            w_t["w1t"] = cpool.tile([128, 4 * D], bf16, tag="w1t", name="w1t")
            nc.sync.dma_start(w_t["w1t"][:], w_ext["w1t"][:])
            dis_t = cpool.tile([128, CH], f32)
            nc.sync.dma_start(dis_t[:], dis_ext[:])
            xt_t = cpool.tile([128, 4 * NPAD], bf16)
            xt_v = xt_t[:].rearrange("p (k n) -> p k n", n=NPAD)
            xe_v = xt_ext[:].rearrange("p (k n) -> p k n", n=NPAD)
            for m in range(CH):
                nc.sync.dma_start(
                    xt_v[:, :, m * 128 : (m + 1) * 128],
                    xe_v[:, :, m * 128 : (m + 1) * 128],
                )
            ident_t = cpool.tile([128, 128], bf16)
            nc.sync.dma_start(ident_t[:], ident_ext[:])
            for nm in ("w2t", "wp1t", "wp2t"):
                w_t[nm] = cpool.tile([128, 4 * D], bf16, tag=nm, name=nm)
                nc.sync.dma_start(w_t[nm][:], w_ext[nm][:])
            b_t = {}
            for nm in b_ext:
                b_t[nm] = cpool.tile([128, D], f32, tag=nm, name=nm + "_bc")
                nc.sync.dma_start(b_t[nm][:], b_ext[nm][:])
            # ---- bulk loads on the scalar HWDGE ring (parallel to above) ----
            idx_t = cpool.tile([128, CH * k_cw * 8], i16)
            nc.scalar.dma_start(idx_t[:], idx_ext[:])
            s8_t = cpool.tile([128, CH * k_cw * 128], f8)
            nc.scalar.dma_start(s8_t[:], s_ext[:])

            # persistent scaled-feature chunks (self-loop term source)
            h1p_t = cpool.tile([128, CH * D], bf16)
            h2p_t = cpool.tile([128, CH * D], bf16)

            def dense_layer(lhs_tiles, w_name, m, dep=None):
                ps = psA.tile([128, D], f32, tag="dense")
                for kk in range(4):
                    mi = nc.tensor.matmul(
                        ps[:],
                        lhs_tiles(kk, m),
                        w_t[w_name][:, kk * D : (kk + 1) * D],
                        start=(kk == 0),
                        stop=(kk == 3),
                    )
                    if kk == 0 and dep is not None:
                        add_dep_helper(
                            getattr(mi, "ins", mi),
                            getattr(dep, "ins", dep),
                            reason="pin proj head to its window",
                        )
                return ps

            def xt_tile(kk, m):
                return xt_t[:, kk * NPAD + m * 128 : kk * NPAD + (m + 1) * 128]

            def scale_to(dst_ap, ps, m, bias_name):
                """dst = dis_m * (ps + bias) via ACT (bias pre-add on DVE).
                Returns the pre-scale source for further ACT copies."""
                if bias_name in b_t:
                    tmp = wpool.tile([128, D], f32, tag="btmp")
                    nc.vector.tensor_tensor(
                        tmp[:], ps[:], b_t[bias_name][:], op=mybir.AluOpType.add
                    )
                    src = tmp
                else:
                    src = ps
                nc.scalar.activation(
                    dst_ap, src[:], ACTF.Copy, scale=dis_t[:, m : m + 1]
                )
                return src

            # ---- phase A: H1' = dis * (X @ W1 + b1), own nodes ----
            for m in range(CH):
                ps = dense_layer(xt_tile, "w1t", m)
                scale_to(h1p_t[:, m * D : (m + 1) * D], ps, m, "b1")
                nc.sync.dma_start(
                    h1p_sh[m * 128 : (m + 1) * 128, :],
                    h1p_t[:, m * D : (m + 1) * D],
                )
                if m == 4:
                    nc.gpsimd.collective_compute(
                        "AllGather",
                        mybir.AluOpType.bypass,
                        ins=[h1p_sh[0:HALF, :]],
                        outs=[h1p_full[0 : NC * HALF, :]],
                        replica_groups=[core_ids],
                    )
            nc.gpsimd.collective_compute(
                "AllGather",
                mybir.AluOpType.bypass,
                ins=[h1p_sh[HALF:NPAD, :]],
                outs=[h1p_full[NC * HALF : 2 * NC * HALF, :]],
                replica_groups=[core_ids],
            )

            def one_gather(src_ap, w, a, nk, g_ap, qn):
                """Gather chunks [a, a+nk) of window w's table into g_ap."""
                gi = nc.gpsimd.dma_gather(
                    g_ap,
                    src_ap,
                    idx_t[
                        :,
                        (w * k_cw + a) * 8 : (w * k_cw + a + nk) * 8,
                    ],
                    num_idxs=nk * 128,
                    num_idxs_reg=nidx_regs[nk],
                    elem_size=D,
                    single_packet=True,
                    queue_num=qn,
                )
                add_dep_helper(
                    getattr(gi, "ins", gi),
                    getattr(lib_inst, "ins", lib_inst),
                    reason="mlp library before dma_gather",
                )

            def issue_gathers(src_full, w, g_t, pieces):
                a = 0
                for hh, nk in enumerate(pieces):
                    one_gather(src_full[:], w, a, nk, g_t[:, a : a + nk, :], hh % 4)
                    a += nk

            def self_add(ps, selfsrc_t, w):
                """Self-loop term on DVE (frees the PE of identity matmuls)."""
                tmp = wpool.tile([128, D], f32, tag="selfadd", bufs=3)
                nc.vector.tensor_tensor(
                    tmp[:],
                    ps[:],
                    selfsrc_t[:, w * D : (w + 1) * D],
                    op=mybir.AluOpType.add,
                )
                return tmp

            def agg_round1(src_full, selfsrc_t):
                """bf16 scatter: per window, DVE-cast the fp8 S chunk to bf16
                then accumulate k_cw chunk matmuls."""
                for w in range(CH):
                    g_t = gpool.tile([128, k_cw, D], bf16, tag="g", name=f"g{w}")
                    issue_gathers(src_full, w, g_t, splits)
                    s_c = scpool.tile(
                        [128, k_cw * 128], bf16, tag="sc", name=f"sc{w}"
                    )
                    nc.vector.tensor_copy(
                        s_c[:], s8_t[:, w * k_cw * 128 : (w + 1) * k_cw * 128]
                    )
                    ps = psB.tile([128, D], f32, tag="agg", name=f"agg{w}")
                    for k in range(k_cw):
                        nc.tensor.matmul(
                            ps[:],
                            s_c[:, k * 128 : (k + 1) * 128],
                            g_t[:, k, :],
                            start=(k == 0),
                            stop=(k == k_cw - 1),
                        )
                    yield w, self_add(ps, selfsrc_t, w)

            s8_pairs = s8_t[:].rearrange("p (c k) -> p c k", k=128)

            def agg_round2(src_full, selfsrc_t, pre=None):
                """fp8 DoubleRow scatter: pairs of 128-src chunks per matmul
                at 2x PE rate; gathers move half the bytes."""
                for w in range(CH):
                    # fp8 tile sized like the bf16 round-1 tiles (2*k_cw
                    # chunks) so both rounds share one pool tag; only the
                    # first k_cw chunks are used.
                    g_full = gpool.tile(
                        [128, 2 * k_cw, D], f8, tag="g", name=f"g8_{w}"
                    )
                    g_t = g_full[:, :k_cw, :]
                    issue_gathers(src_full, w, g_t, splits2)
                    if pre is not None:
                        pre(w)
                    ps = psB.tile([128, D], f32, tag="agg", name=f"agg{w}")
                    np_ = k_cw // 2
                    for c in range(np_):
                        mi = nc.tensor.matmul(
                            ps[:],
                            s8_pairs[:, w * k_cw + 2 * c : w * k_cw + 2 * c + 2, :],
                            g_t[:, 2 * c : 2 * c + 2, :],
                            start=(c == 0),
                            stop=(c == np_ - 1),
                            perf_mode=mybir.MatmulPerfMode.DoubleRow,
                        )
                    yield w, self_add(ps, selfsrc_t, w)

            # ---- fused round 1 + L2, pipelined per window ----
            zt_t = tpool.tile([128, 4 * NPAD], bf16, tag="zt")
            rt_t = tpool.tile([128, 4 * NPAD], bf16, tag="rt")

            def zt_tile(kk, m):
                return zt_t[:, kk * NPAD + m * 128 : kk * NPAD + (m + 1) * 128]

            def rt_tile(kk, m):
                return rt_t[:, kk * NPAD + m * 128 : kk * NPAD + (m + 1) * 128]

            for w, tmp in agg_round1(h1p_full, h1p_t):
                z_b = wpool.tile([128, D], bf16, tag="zb")
                nc.scalar.activation(
                    z_b[:], tmp[:], ACTF.Copy, scale=dis_t[:, w : w + 1]
                )
                nc.sync.dma_start(z_out[w * 128 : (w + 1) * 128, :], z_b[:])
                # transpose z chunk into zt columns; relu'd copy into rt
                psT = psA.tile([128, 4, 128], bf16, tag="tr", name=f"tr{w}")
                for kk in range(4):
                    nc.tensor.transpose(
                        psT[:, kk, :],
                        z_b[:, kk * 128 : (kk + 1) * 128],
                        ident_t[:],
                    )
                zt_cols = zt_t[:].rearrange("p (k n) -> p k n", n=NPAD)[
                    :, :, w * 128 : (w + 1) * 128
                ]
                rt_cols = rt_t[:].rearrange("p (k n) -> p k n", n=NPAD)[
                    :, :, w * 128 : (w + 1) * 128
                ]
                nc.vector.tensor_copy(zt_cols, psT[:])
                nc.vector.tensor_scalar(
                    rt_cols, psT[:], 0.0, None, op0=mybir.AluOpType.max
                )
                # L2 for this node chunk -> H2' shard (bf16 self copy + fp8
                # collective copy)
                ps2 = dense_layer(rt_tile, "w2t", w)
                src2 = scale_to(h2p_t[:, w * D : (w + 1) * D], ps2, w, "b2")
                h2f8 = wpool.tile([128, D], f8, tag="h2f8")
                nc.scalar.activation(
                    h2f8[:], src2[:], ACTF.Copy, scale=dis_t[:, w : w + 1]
                )
                nc.sync.dma_start(
                    h2p_sh[w * 128 : (w + 1) * 128, :], h2f8[:]
                )
                if w == 4:
                    nc.gpsimd.collective_compute(
                        "AllGather",
                        mybir.AluOpType.bypass,
                        ins=[h2p_sh[0:HALF, :]],
                        outs=[h2p_full[0 : NC * HALF, :]],
                        replica_groups=[core_ids],
                    )
            nc.gpsimd.collective_compute(
                "AllGather",
                mybir.AluOpType.bypass,
                ins=[h2p_sh[HALF:NPAD, :]],
                outs=[h2p_full[NC * HALF : 2 * NC * HALF, :]],
                replica_groups=[core_ids],
            )

            # ---- phase E: round 2 -> out, proj head interleaved per window
            # (proj depends only on zt, so its matmuls fill the tensor engine
            # while the h2 AllGather half and the window's gathers land) ----
            def proj_head(w):
                ps3 = dense_layer(zt_tile, "wp1t", w)
                p1_b = wpool.tile([128, D], bf16, tag="p1")
                if "bp1" in b_t:
                    btmp = wpool.tile([128, D], f32, tag="btmp")
                    nc.vector.tensor_tensor(
                        btmp[:], ps3[:], b_t["bp1"][:], op=mybir.AluOpType.add
                    )
                    nc.vector.tensor_scalar(
                        p1_b[:], btmp[:], 0.0, None, op0=mybir.AluOpType.max
                    )
                else:
                    nc.vector.tensor_scalar(
                        p1_b[:], ps3[:], 0.0, None, op0=mybir.AluOpType.max
                    )
                psT2 = psA.tile([128, 4, 128], bf16, tag="tr", name=f"tr2{w}")
                for kk in range(4):
                    nc.tensor.transpose(
                        psT2[:, kk, :],
                        p1_b[:, kk * 128 : (kk + 1) * 128],
                        ident_t[:],
                    )
                # p1^T is consumed immediately by proj2 — rotating tile, not
                # a persistent NPAD-wide buffer
                p1c = scpool.tile([128, 4, 128], bf16, tag="p1c", name=f"p1c{w}")
                nc.vector.tensor_copy(p1c[:], psT2[:])
                ps4 = dense_layer(lambda kk, m: p1c[:, kk, :], "wp2t", w)
                pj_t = wpool.tile([128, D], f32, tag="pj")
                if "bp2" in b_t:
                    nc.vector.tensor_tensor(
                        pj_t[:], ps4[:], b_t["bp2"][:], op=mybir.AluOpType.add
                    )
                else:
                    nc.vector.tensor_copy(pj_t[:], ps4[:])
                nc.sync.dma_start(proj_out[w * 128 : (w + 1) * 128, :], pj_t[:])

            for w, tmp in agg_round2(h2p_full, h2p_t, pre=proj_head):
                o_f = wpool.tile([128, D], f32, tag="of")
                nc.scalar.activation(
                    o_f[:], tmp[:], ACTF.Copy, scale=dis_t[:, w : w + 1]
                )
                nc.sync.dma_start(out_out[w * 128 : (w + 1) * 128, :], o_f[:])

    lower_extended_insts(nc)
    return nc


def _host_prep(x, edge_index, W1, W2, Wp1, Wp2):
    src = np.asarray(edge_index[0], np.int64)
    dst = np.asarray(edge_index[1], np.int64)

    # degree includes self loops (norm definition), but self edges are
    # handled on-device via the identity matmul, not the gather.
    deg = (np.bincount(np.concatenate([dst, np.arange(N)]), minlength=N)).astype(
        np.float32
    )
    dis = (1.0 / np.sqrt(np.maximum(deg, 1.0))).astype(np.float32)

    owner = src // NPC
    local = src - owner * NPC
    HALF = NPAD // 2
    # AllGather halves land rank-major per half: [8*640 | 8*640]
    gather_row = np.where(
        local < HALF,
        owner * HALF + local,
        NC * HALF + owner * HALF + (local - HALF),
    )

    dst_core = dst // NPC
    dst_local = dst - dst_core * NPC  # [0, 1250)
    win = dst_local // 128
    dloc = dst_local - win * 128

    order = np.lexsort((dst_local, dst_core))
    g_sorted = gather_row[order]
    dc = dst_core[order]
    wn = win[order]
    dl = dloc[order]

    counts = np.zeros((NC, CH), np.int64)
    np.add.at(counts, (dc, wn), 1)
    flat_counts = counts.reshape(-1)
    starts = np.concatenate([[0], np.cumsum(flat_counts)])[:-1].reshape(NC, CH)

    # dedup per (core, window); k_cw from max unique count
    uniq = {}
    max_u = 0
    for c in range(NC):
        for w in range(CH):
            s0, n = starts[c, w], counts[c, w]
            rows = g_sorted[s0 : s0 + n]
            dd = dl[s0 : s0 + n]
            u, inv = np.unique(rows, return_inverse=True)
            uniq[(c, w)] = (u, inv, dd)
            max_u = max(max_u, len(u))
    k_cw = int(np.ceil(max_u / 128))
    if k_cw % 2:
        k_cw += 1  # even: round-2 fp8 DoubleRow consumes chunk pairs
    wlen = k_cw * 128

    # Unique sources are sorted ascending, and AllGather half A lands in
    # rows [0, NC*HALF): the first cntA entries of each window's table
    # depend only on AG half A. Per window, the number of 2-chunk gather
    # pieces that are A-only on EVERY core (shared program) lets round-1
    # start those pieces as soon as AG0 lands (overlapping AG1).
    HALF = NPAD // 2
    nA_pieces = []
    for w in range(CH):
        m = min(
            int(np.searchsorted(uniq[(c, w)][0], NC * HALF)) for c in range(NC)
        )
        nA_pieces.append(min(m // 256, k_cw // 2 - 1))

    per_core = []
    for c in range(NC):
        idx_pad = np.zeros((CH, wlen), np.int64)
        s_tab = np.zeros((CH, wlen, 128), np.float32)
        for w in range(CH):
            u, inv, dd = uniq[(c, w)]
            idx_pad[w, : len(u)] = u
            np.add.at(s_tab[w], (inv, dd), 1.0)

        iw = idx_pad.reshape(CH, wlen // 16, 16).transpose(0, 2, 1)
        idx16 = np.tile(iw, (1, 8, 1)).transpose(1, 0, 2).reshape(128, -1)
        idx16 = np.ascontiguousarray(idx16, np.int16)

        # stab: [128, CH*k_cw*128]; col (w*k_cw+k)*128+d, part p = S[w, k*128+p, d]
        # fp8e4: counts are small ints, exactly representable
        stab = (
            s_tab.reshape(CH, k_cw, 128, 128)
            .transpose(2, 0, 1, 3)
            .reshape(128, -1)
        )
        stab = np.ascontiguousarray(stab).astype(_F8)

        xc = np.zeros((NPAD, D), np.float32)
        xc[:NPC] = x[c * NPC : (c + 1) * NPC]
        xt = xc.T.reshape(4, 128, NPAD).transpose(1, 0, 2).reshape(128, -1)
        xt = np.ascontiguousarray(xt).astype(_BF16)

        dis_c = np.zeros((NPAD,), np.float32)
        dis_c[:NPC] = dis[c * NPC : (c + 1) * NPC]
        dis_t = np.ascontiguousarray(dis_c.reshape(CH, 128).T, np.float32)

        per_core.append(
            {"xt": xt, "idx16": idx16, "stab": stab, "dis": dis_t}
        )

    def wtile(W):
        wt = (
            np.asarray(W, np.float32)
            .reshape(4, 128, D)
            .transpose(1, 0, 2)
            .reshape(128, -1)
        )
        return np.ascontiguousarray(wt).astype(_BF16)

    shared = {
        "w1t": wtile(W1),
        "w2t": wtile(W2),
        "wp1t": wtile(Wp1),
        "wp2t": wtile(Wp2),
        "ident": np.eye(128, dtype=np.float32).astype(_BF16),
    }
    return k_cw, nA_pieces, per_core, shared


def run(inputs, trace=False, **run_kwargs):
    """Build + run; returns ((out, z, proj), BassKernelResults)."""
    _install_wait_split()
    from concourse.bass_utils import run_bass_kernel_spmd

    x = np.asarray(inputs["x"], np.float32)
    b1, b2 = inputs["b1"], inputs["b2"]
    bp1, bp2 = inputs["bp1"], inputs["bp2"]
    k_cw, nA_pieces, per_core, shared = _host_prep(
        x, inputs["edge_index"], inputs["W1"], inputs["W2"], inputs["Wp1"],
        inputs["Wp2"],
    )

    has_b = {
        "b1": bool(np.any(np.asarray(b1))),
        "b2": bool(np.any(np.asarray(b2))),
        "bp1": bool(np.any(np.asarray(bp1))),
        "bp2": bool(np.any(np.asarray(bp2))),
    }
    nc = _build_program(
        k_cw, nA_pieces, has_b["b1"], has_b["b2"], has_b["bp1"], has_b["bp2"]
    )

    in_maps = []
    for c in range(NC):
        m = dict(per_core[c])
        m.update(shared)
        for nm, b in (("b1", b1), ("b2", b2), ("bp1", bp1), ("bp2", bp2)):
            if has_b[nm]:
                m[nm] = np.ascontiguousarray(
                    np.tile(np.asarray(b, np.float32)[None, :], (128, 1))
                )
        in_maps.append(m)

    res = run_bass_kernel_spmd(
        nc, in_maps, core_ids=list(range(NC)), trace=trace, **run_kwargs
    )

    out = np.empty((N, D), np.float32)
    z = np.empty((N, D), np.float32)
    proj = np.empty((N, D), np.float32)
    for c in range(NC):
        r = res.results[c]
        out[c * NPC : (c + 1) * NPC] = r["agg"][:NPC]
        z[c * NPC : (c + 1) * NPC] = r["z"][:NPC]
        proj[c * NPC : (c + 1) * NPC] = r["proj"][:NPC]
    return (out, z, proj), res


def kernel(x, edge_index, W1, b1, W2, b2, Wp1, bp1, Wp2, bp2):
    outs, _ = run(
        {
            "x": x, "edge_index": edge_index, "W1": W1, "b1": b1,
            "W2": W2, "b2": b2, "Wp1": Wp1, "bp1": bp1,
            "Wp2": Wp2, "bp2": bp2,
        }
    )
    return outs

